# revision 24
# baseline (speedup 1.0000x reference)
"""ChildSumTreeLSTM with relation transforms on 8 Trainium2 NeuronCores.

Layout: transposed (features on SBUF partitions, tree nodes on the free dim),
feature-sharded state (each core owns a 128-feature slice of h/c/xi/gates).

Column order: the leaf region is laid out in wave-1 child-slot order
(child k of wave-1 grouped parent t sits at column 4t+k, missing/pad slots
are dummy zero columns) so wave-1's child gather is a plain view.  Each
sharded internal wave's parent columns are grouped by the core that owns
their relation (8 equal padded groups) so every exchange is an AllGather
of the core's own contiguous shard — no AllReduce anywhere:

  per sharded wave: hsum -> per-slot (rel) masked-rhs matmuls accumulated
  into one PSUM region (mask selects that rel's columns; the per-core psum
  is then nonzero only on the core's own column group) -> group-axis
  reduce packs the AG shard -> AllGather ch_sum -> column-sharded iou
  gates + f gates -> AllGather of the new h feature-slices.

The tiny top waves (3, 4, 5 — 8 nodes) run fully REPLICATED on every core
(full-feature compute, relation weights replicated, host-precomputed xi),
eliminating their collectives; each core extracts its own feature slice of
the results with a data-driven select mask.  AGh2 additionally publishes
the c state those waves need.  Relation weights are fp8 (x WSCALE), with
1/WSCALE folded into the iouh weights / activation scales.  A warmup
collective at t~0 absorbs the ~52us collectives-firmware spin-up.
All per-core differences are input data, so one Bass program runs SPMD.
"""

import sys

sys.path.insert(0, "/opt/trn_rl_repo")

import numpy as np
import ml_dtypes

import concourse.bass as bass
import concourse.mybir as mybir
import concourse.tile as tile
from concourse.bass_utils import run_bass_kernel_spmd
from concourse.vector_clock import ScopedClock, VectorClock

BF16 = mybir.dt.bfloat16
FP8 = mybir.dt.float8e4
F32 = mybir.dt.float32
NCORES = 8
P = 128
WSCALE = 64.0   # fp8 weights are stored x WSCALE
REPLF_MAX = 8   # waves with <= this many cols run replicated on all cores


def _split_drain_and_barrier(self, tick_clock, wait_clock):
    gc = tick_clock.global_clock
    n = len(gc)
    nonzero = [i for i in range(n) if gc[i] > 0]
    for j in nonzero:
        vec = VectorClock([gc[i] if i == j else 0 for i in range(n)])
        d = self.nc.sync.drain()
        wait_clock.add_sem_waits(d.ins, ScopedClock({None: vec}))
    if not nonzero:
        d = self.nc.sync.drain()
        wait_clock.add_sem_waits(d.ins, ScopedClock({None: gc.copy()}))
    self.nc.all_engine_barrier()
    assert self.sems is not None
    popped = self.nc._tile_sem_poison_stack.pop()
    assert popped is self._sem_poison
    self.nc.clear_and_free_semaphores(list(self.sems.allocated().values()))
    self.nc.all_engine_barrier()


tile.TileContext._drain_and_barrier = _split_drain_and_barrier


def _split_multi_waits(nc, limit=1):
    for bb in nc.main_func.blocks:
        new_list = []
        for ins in bb.instructions:
            si = getattr(ins, "sync_info", None)
            if si is not None and si.on_wait and len(si.on_wait) > limit:
                waits = list(si.on_wait)
                for w in waits[:-limit]:
                    nop = mybir.InstNoOp(
                        name=nc.get_next_instruction_name(),
                        sync_info=mybir.SyncInfo(on_wait=[w], on_update=[]),
                        bass_nofuse=True,
                        engine=ins.engine,
                    )
                    nc.register_instruction(nop, overwrite=True)
                    new_list.append(nop)
                si.on_wait = waits[-limit:]
            new_list.append(ins)
        bb.instructions[:] = new_list


def _bf16(a):
    return np.ascontiguousarray(a.astype(ml_dtypes.bfloat16))


def _fp8w(a):
    return (np.asarray(a, np.float32) * WSCALE).astype(ml_dtypes.float8_e4m3)


def _blocksT(mat):
    """[M, K] -> [M/128 * K/128, 128, 128]; index m*KC+k holds mat[mb,kb].T"""
    M, K = mat.shape
    MC, KC = M // P, K // P
    out = np.empty((MC * KC, P, P), mat.dtype)
    for m in range(MC):
        for k in range(KC):
            out[m * KC + k] = mat[m * P:(m + 1) * P, k * P:(k + 1) * P].T
    return out


def _runs(seq, zcol):
    runs = []
    i0 = 0
    n = len(seq)
    while i0 < n:
        if seq[i0] == zcol:
            i0 += 1
            continue
        i1 = i0 + 1
        while i1 < n and seq[i1] == seq[i1 - 1] + 1 and seq[i1] != zcol:
            i1 += 1
        runs.append((i0, int(seq[i0]), i1 - i0))
        i0 = i1
    return runs


def _plan(child_idx, rel_ids, Wrel):
    N, K = child_idx.shape
    eff_children = []
    wave = np.zeros(N, np.int32)
    for i in range(N):
        cs = [int(c) for c in child_idx[i] if 0 <= c < i]
        eff_children.append(cs)
        wave[i] = 1 + max((wave[c] for c in cs), default=-1)
    nwaves = int(wave.max()) + 1
    wave_nodes = [sorted([i for i in range(N) if wave[i] == w],
                         key=lambda i: -i) for w in range(nwaves)]

    from collections import Counter
    wave_info = []
    for w in range(1, nwaves):
        nodes = wave_nodes[w]
        if len(nodes) <= REPLF_MAX:
            # replicated-full wave: order cols by (rel, -node); per-rel ranges
            grouped = sorted(nodes, key=lambda i: (int(rel_ids[i]), -i))
            ranges = []  # (rel, lo, hi)
            for t, node in enumerate(grouped):
                r = int(rel_ids[node])
                if ranges and ranges[-1][0] == r:
                    ranges[-1][2] = t + 1
                else:
                    ranges.append([r, t, t + 1])
            wave_info.append(dict(kind="replf", nodes=nodes, grouped=grouped,
                                  ncols=len(grouped), ranges=ranges))
        else:
            cnt = Counter(int(rel_ids[i]) for i in nodes)
            rels = sorted(cnt, key=lambda r: -cnt[r])
            ns = (len(rels) + NCORES - 1) // NCORES
            core_rels = [[] for _ in range(NCORES)]
            core_cols = [0] * NCORES
            for r in rels:
                best = min(range(NCORES),
                           key=lambda c: (len(core_rels[c]) >= ns,
                                          core_cols[c], len(core_rels[c])))
                core_rels[best].append(r)
                core_cols[best] += cnt[r]
            nmax = max(core_cols)
            grouped = []
            for c in range(NCORES):
                cn = [i for i in nodes if int(rel_ids[i]) in core_rels[c]]
                cn.sort(key=lambda i: (int(rel_ids[i]), -i))
                grouped.extend(cn)
                grouped.extend([None] * (nmax - len(cn)))
            wave_info.append(dict(kind="shard", nodes=nodes, ns=ns,
                                  core_rels=core_rels, nmax=nmax,
                                  grouped=grouped, ncols=NCORES * nmax))

    # ---- global column order ------------------------------------------------
    # leaf region: wave-1 child-slot order (4 slots per wave-1 grouped col),
    # then any leaves that are not children of wave-1 parents.
    col_of = np.full(N, -1, np.int64)
    w1 = wave_info[0]
    leafdata = []  # node or None per col
    for node in w1["grouped"]:
        cs = eff_children[node] if node is not None else []
        leafdata.extend(cs + [None] * (K - len(cs)))
    placed = set(c for c in leafdata if c is not None)
    orphans = [i for i in wave_nodes[0] if i not in placed]
    leafdata.extend(orphans)
    for j, node in enumerate(leafdata):
        if node is not None:
            col_of[node] = j
    order_cols = list(leafdata)
    n0 = len(order_cols)
    for info in wave_info:
        info["base"] = len(order_cols)
        for node in info["grouped"]:
            if node is not None:
                col_of[node] = len(order_cols)
            order_cols.append(node)
    C = len(order_cols)
    ZCOL = C
    NPAD = C + 6

    # ---- child gather runs (wave 1 is a pure view of the leaf region) ------
    for wi, info in enumerate(wave_info):
        if wi == 0:
            info["view0"] = True
            info["runs"] = []
            info["has_missing"] = False
            continue
        info["view0"] = False
        seq = []
        for node in info["grouped"]:
            if node is None:
                seq.extend([ZCOL] * K)
            else:
                cs = eff_children[node]
                seq.extend([int(col_of[c]) for c in cs])
                seq.extend([ZCOL] * (K - len(cs)))
        info["runs"] = _runs(seq, ZCOL)
        info["has_missing"] = any(s == ZCOL for s in seq)
        info["child_cols"] = [s for s in seq]

    # ---- c-state publication plan for the replicated waves ------------------
    # cfull positions: [last-shard-wave cols + extras] ++ [replf waves' cols]
    shard_is = [i for i, x in enumerate(wave_info) if x["kind"] == "shard"]
    repl_is = [i for i, x in enumerate(wave_info) if x["kind"] == "replf"]
    lastsh = wave_info[shard_is[-1]]
    repl_colset = set()
    for i in repl_is:
        info = wave_info[i]
        repl_colset.update(range(info["base"], info["base"] + info["ncols"]))
    need_c = set()
    for i in repl_is:
        for s in wave_info[i]["child_cols"]:
            if s != ZCOL and s not in repl_colset:
                need_c.add(s)
    sh_cols = set(range(lastsh["base"], lastsh["base"] + lastsh["ncols"]))
    extras = sorted(need_c - sh_cols)
    pub_cols = list(range(lastsh["base"], lastsh["base"] + lastsh["ncols"])) \
        + extras
    cpos_of = {c: t for t, c in enumerate(pub_cols)}
    t = len(pub_cols)
    for i in repl_is:
        info = wave_info[i]
        info["cpos0"] = t
        for cc in range(info["base"], info["base"] + info["ncols"]):
            cpos_of[cc] = t
            t += 1
    CFN = t
    lastsh["extras"] = extras
    # replf gather runs in cfull coordinates
    for i in repl_is:
        info = wave_info[i]
        cseq = [cpos_of.get(s, -1) if s != ZCOL else -1
                for s in info["child_cols"]]
        assert all(p >= 0 for p, s in zip(cseq, info["child_cols"])
                   if s != ZCOL)
        info["cruns"] = _runs([p if s != ZCOL else CFN
                               for p, s in zip(cseq, info["child_cols"])], CFN)

    return dict(wave_nodes=wave_nodes, wave_info=wave_info, col_of=col_of,
                order_cols=order_cols, C=C, ZCOL=ZCOL, NPAD=NPAD, n0=n0,
                CFN=CFN, eff_children=eff_children)


def kernel(**inputs):
    x = np.asarray(inputs["x"], np.float32)
    Wrel = np.asarray(inputs["Wrel"], np.float32)
    ioux_w = np.asarray(inputs["ioux_w"], np.float32)
    ioux_b = np.asarray(inputs["ioux_b"], np.float32)
    iouh_w = np.asarray(inputs["iouh_w"], np.float32)
    iouh_b = np.asarray(inputs["iouh_b"], np.float32)
    fx_w = np.asarray(inputs["fx_w"], np.float32)
    fx_b = np.asarray(inputs["fx_b"], np.float32)
    fh_w = np.asarray(inputs["fh_w"], np.float32)
    fh_b = np.asarray(inputs["fh_b"], np.float32)
    child_idx = np.asarray(inputs["child_idx"], np.int32)
    rel_ids = np.asarray(inputs["rel_ids"], np.int32)

    # dummy leaf slots rely on zero iou biases producing h == c == 0
    assert np.allclose(ioux_b, 0) and np.allclose(iouh_b, 0)

    N, IN_DIM = x.shape
    MEM = fh_w.shape[0]
    KC = MEM // P
    KX = IN_DIM // P
    K = child_idx.shape[1]

    plan = _plan(child_idx, rel_ids, Wrel)
    wave_info, col_of = plan["wave_info"], plan["col_of"]
    order_cols, C, NPAD = plan["order_cols"], plan["C"], plan["NPAD"]
    n0, CFN = plan["n0"], plan["CFN"]

    # ---- per-core host data -------------------------------------------------
    xT = np.zeros((IN_DIM, C), np.float32)
    for j, node in enumerate(order_cols):
        if node is not None:
            xT[:, j] = x[node]
    xT_b = np.zeros((KX, P, C), ml_dtypes.bfloat16)
    for k in range(KX):
        xT_b[k] = _bf16(xT[k * P:(k + 1) * P])

    shard_waves = [i for i in wave_info if i["kind"] == "shard"]
    repl_waves = [i for i in wave_info if i["kind"] == "replf"]
    S_total = sum(i["ns"] for i in shard_waves) \
        + sum(len(i["ranges"]) for i in repl_waves)
    MTK = sum(i["ns"] * KC * i["ncols"] for i in shard_waves)

    wstream = [np.zeros((S_total, P, KC * KC, P), ml_dtypes.float8_e4m3)
               for _ in range(NCORES)]
    maskbuf = [np.zeros((P, max(MTK, 1)), ml_dtypes.bfloat16)
               for _ in range(NCORES)]
    eyeW = np.eye(MEM, dtype=np.float32)
    soff = 0
    moff = 0
    for info in wave_info:
        if info["kind"] == "shard":
            ncols = info["ncols"]
            info["soff"], info["moff"] = soff, moff
            for c in range(NCORES):
                for s, r in enumerate(info["core_rels"][c]):
                    wstream[c][soff + s] = _fp8w(
                        _blocksT(Wrel[r]).transpose(1, 0, 2))
                    m = np.zeros((KC, ncols), np.float32)
                    for t in range(ncols):
                        node = info["grouped"][t]
                        if node is not None and int(rel_ids[node]) == r:
                            m[:, t] = 1.0
                    mo = moff + s * KC * ncols
                    maskbuf[c][:, mo:mo + KC * ncols] = _bf16(
                        np.broadcast_to(m.reshape(1, -1), (P, KC * ncols)))
            soff += info["ns"]
            moff += info["ns"] * KC * ncols
        else:
            info["soff"] = soff
            for (r, lo, hi) in info["ranges"]:
                Wm = Wrel[r] if r < Wrel.shape[0] else eyeW
                wb = _fp8w(_blocksT(Wm).transpose(1, 0, 2))
                for c in range(NCORES):
                    wstream[c][soff] = wb
                soff += 1

    # full-feature constants for the replicated waves (fp8 x WSCALE);
    # block order: [g*KC + mchunk] for iouh, [m*KC + k] plain for fh
    iouhF = np.zeros((3 * KC * KC, P, P), ml_dtypes.float8_e4m3)
    bT = _blocksT(iouh_w)  # [(3KC)*KC]
    for mg in range(3 * KC):
        for k in range(KC):
            iouhF[mg * KC + k] = _fp8w(bT[mg * KC + k])
    fhF = np.zeros((KC * KC, P, P), ml_dtypes.float8_e4m3)
    bT = _blocksT(fh_w)
    for i in range(KC * KC):
        fhF[i] = _fp8w(bT[i])
    # host xi / xf for replicated cols (true scale; biases folded in)
    NF = sum(i["ncols"] for i in repl_waves)
    xiF = np.zeros((3 * KC, P, max(NF, 1)), np.float32)
    xffF = np.zeros((KC, P, max(NF, 1)), np.float32)
    selm = [np.zeros((P, KC, max(NF, 1)), np.float32) for _ in range(NCORES)]
    off = 0
    for info in repl_waves:
        info["foff"] = off
        for t, node in enumerate(info["grouped"]):
            xi_full = ioux_w @ x[node] + ioux_b + iouh_b  # [3*MEM]
            xf_full = (fx_w @ x[node] + fx_b + fh_b) * WSCALE
            for g in range(3):
                for m in range(KC):
                    xiF[g * KC + m, :, off + t] = \
                        xi_full[g * MEM + m * P:(g * MEM + (m + 1) * P)]
            for m in range(KC):
                xffF[m, :, off + t] = xf_full[m * P:(m + 1) * P]
        off += info["ncols"]
    for c in range(NCORES):
        selm[c][:, c, :] = 1.0

    iouxstat = [np.zeros((KX * 3, P, P), ml_dtypes.bfloat16)
                for _ in range(NCORES)]
    iouhstat = [np.zeros((KC * 3, P, P), ml_dtypes.bfloat16)
                for _ in range(NCORES)]
    fxstat = [np.zeros((KX, P, P), ml_dtypes.bfloat16) for _ in range(NCORES)]
    fhstat = [np.zeros((KC, P, P), ml_dtypes.bfloat16) for _ in range(NCORES)]
    b_xi = [np.zeros((3, P), np.float32) for _ in range(NCORES)]
    b_iou = [np.zeros((3, P), np.float32) for _ in range(NCORES)]
    b_xf = [np.zeros((P,), np.float32) for _ in range(NCORES)]
    b_fh = [np.zeros((P,), np.float32) for _ in range(NCORES)]
    for c in range(NCORES):
        rows = slice(c * P, (c + 1) * P)
        for g in range(3):
            gr = slice(g * MEM + c * P, g * MEM + (c + 1) * P)
            b_xi[c][g] = ioux_b[gr]
            b_iou[c][g] = iouh_b[gr]
            for k in range(KX):
                iouxstat[c][k * 3 + g] = _bf16(ioux_w[gr, k * P:(k + 1) * P].T)
            for k in range(KC):
                iouhstat[c][k * 3 + g] = _bf16(
                    iouh_w[gr, k * P:(k + 1) * P].T / WSCALE)
        b_xf[c] = fx_b[rows]
        b_fh[c] = fh_b[rows]
        for k in range(KX):
            fxstat[c][k] = _bf16(fx_w[rows, k * P:(k + 1) * P].T)
        for k in range(KC):
            fhstat[c][k] = _bf16(fh_w[rows, k * P:(k + 1) * P].T)

    # ---- build program ------------------------------------------------------
    nc = bass.Bass("TRN2", target_bir_lowering=False, debug=False,
                   num_devices=NCORES)
    d_ws = nc.dram_tensor("wstream", list(wstream[0].shape), FP8,
                          kind="ExternalInput")
    d_mask = nc.dram_tensor("masks", list(maskbuf[0].shape), BF16,
                            kind="ExternalInput")
    d_xt = nc.dram_tensor("xt", [KX, P, C], BF16, kind="ExternalInput")
    d_iouxs = nc.dram_tensor("iouxstat", [KX * 3, P, P], BF16,
                             kind="ExternalInput")
    d_iouhs = nc.dram_tensor("iouhstat", [KC * 3, P, P], BF16,
                             kind="ExternalInput")
    d_fxs = nc.dram_tensor("fxstat", [KX, P, P], BF16, kind="ExternalInput")
    d_fhs = nc.dram_tensor("fhstat", [KC, P, P], BF16, kind="ExternalInput")
    d_iouhF = nc.dram_tensor("iouhF", [3 * KC * KC, P, P], FP8,
                             kind="ExternalInput")
    d_fhF = nc.dram_tensor("fhF", [KC * KC, P, P], FP8, kind="ExternalInput")
    d_xiF = nc.dram_tensor("xiF", [3 * KC, P, max(NF, 1)], F32,
                           kind="ExternalInput")
    d_xffF = nc.dram_tensor("xffF", [KC, P, max(NF, 1)], F32,
                            kind="ExternalInput")
    d_selm = nc.dram_tensor("selm", [P, KC, max(NF, 1)], F32,
                            kind="ExternalInput")
    d_bxi = nc.dram_tensor("b_xi", [3, P], F32, kind="ExternalInput")
    d_biou = nc.dram_tensor("b_iou", [3, P], F32, kind="ExternalInput")
    d_bxf = nc.dram_tensor("b_xf", [P], F32, kind="ExternalInput")
    d_bfh = nc.dram_tensor("b_fh", [P], F32, kind="ExternalInput")
    d_hout = nc.dram_tensor("hout", [P, C], F32, kind="ExternalOutput")

    ACT = mybir.ActivationFunctionType
    NMAXC = max([i["ncols"] for i in shard_waves] + [1])
    NCHMX = K * NMAXC
    NFC = max(NF, 1)

    with tile.TileContext(nc, num_cores=NCORES) as tc:
        with (
            tc.tile_pool(name="const", bufs=1) as cpool,
            tc.tile_pool(name="state", bufs=1) as spool,
            tc.tile_pool(name="wstage", bufs=8) as wpool,
            tc.tile_pool(name="work", bufs=1) as wk,
            tc.tile_pool(name="psum", bufs=1, space="PSUM") as pp,
            tc.tile_pool(name="psg", bufs=1, space="PSUM") as pg,
            tc.tile_pool(name="dram", bufs=2, space="DRAM") as dp,
        ):
            # ---- warmup collective: absorbs the CC firmware spin-up --------
            wu = cpool.tile([P, 8], BF16)
            nc.vector.memset(wu[:], 0.0)
            wu_in = dp.tile([P, 8], BF16, tag="wu_in")
            nc.scalar.dma_start(wu_in[:], wu[:])
            wu_out = dp.tile([NCORES, P, 8], BF16, tag="wu_out",
                             addr_space="Shared")
            nc.gpsimd.collective_compute(
                "AllGather", mybir.AluOpType.bypass,
                ins=[wu_in.opt()], outs=[wu_out.opt()],
                replica_groups=[list(range(NCORES))])

            # ---- constants (SP queue) --------------------------------------
            xt = cpool.tile([P, KX, C], BF16)
            nc.sync.dma_start(xt[:], d_xt.ap().rearrange("k p n -> p k n"))
            iouxs = cpool.tile([P, KX * 3, P], BF16)
            nc.sync.dma_start(iouxs[:], d_iouxs.ap().rearrange("s p m -> p s m"))
            iouhs = cpool.tile([P, KC * 3, P], BF16)
            nc.sync.dma_start(iouhs[:], d_iouhs.ap().rearrange("s p m -> p s m"))
            fxs = cpool.tile([P, KX, P], BF16)
            nc.sync.dma_start(fxs[:], d_fxs.ap().rearrange("s p m -> p s m"))
            fhs = cpool.tile([P, KC, P], BF16)
            nc.sync.dma_start(fhs[:], d_fhs.ap().rearrange("s p m -> p s m"))
            bxi = cpool.tile([P, 3], F32)
            nc.sync.dma_start(bxi[:], d_bxi.ap().rearrange("g p -> p g"))
            biou = cpool.tile([P, 3], F32)
            nc.sync.dma_start(biou[:], d_biou.ap().rearrange("g p -> p g"))
            bxf = cpool.tile([P, 1], F32)
            nc.sync.dma_start(bxf[:], d_bxf.ap().rearrange(
                "(p one) -> p one", one=1))
            bfh = cpool.tile([P, 1], F32)
            nc.sync.dma_start(bfh[:], d_bfh.ap().rearrange(
                "(p one) -> p one", one=1))
            mbuf = cpool.tile([P, max(MTK, 1)], BF16)
            nc.sync.dma_start(mbuf[:], d_mask.ap())
            xiFt = cpool.tile([P, 3 * KC, NFC], F32)
            nc.sync.dma_start(xiFt[:], d_xiF.ap().rearrange("s p n -> p s n"))
            xffFt = cpool.tile([P, KC, NFC], F32)
            nc.sync.dma_start(xffFt[:], d_xffF.ap().rearrange("s p n -> p s n"))
            selmt = cpool.tile([P, KC, NFC], F32)
            nc.sync.dma_start(selmt[:], d_selm.ap())

            # ---- bulk weights (ACT queue; never blocks the SP queue) -------
            wtile = []
            for s in range(S_total):
                t = wpool.tile([P, KC * KC, P], FP8, tag="wst")
                nc.scalar.dma_start(t[:], d_ws.ap()[s])
                wtile.append(t)
            iouhFt = cpool.tile([P, 3 * KC * KC, P], FP8)
            nc.scalar.dma_start(iouhFt[:],
                                d_iouhF.ap().rearrange("s p m -> p s m"))
            fhFt = cpool.tile([P, KC * KC, P], FP8)
            nc.scalar.dma_start(fhFt[:], d_fhF.ap().rearrange("s p m -> p s m"))

            # ---- state -----------------------------------------------------
            h_bf = spool.tile([P, KC, NPAD], BF16)
            nc.vector.memset(h_bf[:], 0.0)
            c_sl = spool.tile([P, NPAD], F32)
            nc.vector.memset(c_sl[:], 0.0)
            h_sl = spool.tile([P, C], F32)
            xi_f = spool.tile([P, 3, C], F32)
            xf_f = spool.tile([P, C], F32)
            cfull = spool.tile([P, KC, CFN + 2], BF16)

            # ---- xi/xf precompute (feature-sharded) ------------------------
            CCH = 128
            for cc in range(0, C, CCH):
                ncc = min(CCH, C - cc)
                ps = pg.tile([P, 3, CCH], F32, tag="ps3")
                for g in range(3):
                    for k in range(KX):
                        nc.tensor.matmul(
                            ps[:, g, :ncc], iouxs[:, k * 3 + g, :],
                            xt[:, k, cc:cc + ncc],
                            start=(k == 0), stop=(k == KX - 1))
                for g in range(3):
                    nc.scalar.activation(
                        xi_f[:, g, cc:cc + ncc], ps[:, g, :ncc],
                        ACT.Identity, bias=bxi[:, g:g + 1])
                psf0 = pg.tile([P, CCH], F32, tag="psf0")
                for k in range(KX):
                    nc.tensor.matmul(
                        psf0[:, :ncc], fxs[:, k, :], xt[:, k, cc:cc + ncc],
                        start=(k == 0), stop=(k == KX - 1))
                nc.scalar.activation(
                    xf_f[:, cc:cc + ncc], psf0[:, :ncc],
                    ACT.Identity, bias=bxf[:, 0:1])

            def publish(p0, ncols, tag, c_extra=None):
                """AllGather this core's h feature-slice for cols
                [p0, p0+ncols); optionally also c for those cols plus the
                c_extra col list (writes cfull[0:. ] in publish order)."""
                rowdim = 2 if c_extra is not None else 1
                nce = ncols + (len(c_extra) if c_extra else 0)
                hb = wk.tile([P, rowdim, nce], BF16, tag="hb" + tag)
                nc.vector.tensor_copy(hb[:, 0, :ncols], h_sl[:, p0:p0 + ncols])
                if c_extra is not None:
                    nc.vector.tensor_copy(hb[:, 1, :ncols],
                                          c_sl[:, p0:p0 + ncols])
                    for j, ecol in enumerate(c_extra):
                        nc.vector.tensor_copy(hb[:, 0, ncols + j:ncols + j + 1],
                                              h_sl[:, ecol:ecol + 1])
                        nc.vector.tensor_copy(hb[:, 1, ncols + j:ncols + j + 1],
                                              c_sl[:, ecol:ecol + 1])
                gin = dp.tile([P, rowdim, nce], BF16, tag="gin" + tag)
                nc.sync.dma_start(gin[:], hb[:])
                gout = dp.tile([NCORES, P, rowdim, nce], BF16,
                               tag="gout" + tag, addr_space="Shared")
                nc.gpsimd.collective_compute(
                    "AllGather", mybir.AluOpType.bypass,
                    ins=[gin.opt()], outs=[gout.opt()],
                    replica_groups=[list(range(NCORES))])
                nc.sync.dma_start(
                    h_bf[:, :, p0:p0 + ncols],
                    gout[:, :, 0, :ncols].rearrange("k p n -> p k n"))
                if c_extra is not None:
                    nc.sync.dma_start(
                        cfull[:, :, 0:nce],
                        gout[:, :, 1, :].rearrange("k p n -> p k n"))

            # ---- wave 0: leaves (iou = xi, no children) --------------------
            for cc in range(0, n0, NMAXC):
                L = min(NMAXC, n0 - cc)
                ig = wk.tile([P, NMAXC], F32, tag="ig")
                og = wk.tile([P, NMAXC], F32, tag="og")
                ug = wk.tile([P, NMAXC], F32, tag="ug")
                nc.scalar.activation(ig[:, :L], xi_f[:, 0, cc:cc + L],
                                     ACT.Sigmoid, bias=biou[:, 0:1])
                nc.scalar.activation(og[:, :L], xi_f[:, 1, cc:cc + L],
                                     ACT.Sigmoid, bias=biou[:, 1:2])
                nc.scalar.activation(ug[:, :L], xi_f[:, 2, cc:cc + L],
                                     ACT.Tanh, bias=biou[:, 2:3])
                nc.vector.tensor_mul(c_sl[:, cc:cc + L], ig[:, :L], ug[:, :L])
                tct = wk.tile([P, NMAXC], F32, tag="tct")
                nc.scalar.activation(tct[:, :L], c_sl[:, cc:cc + L], ACT.Tanh)
                nc.vector.tensor_mul(h_sl[:, cc:cc + L], og[:, :L], tct[:, :L])
            publish(0, n0, "0")

            # ---- sharded internal waves ------------------------------------
            for wi, info in enumerate(wave_info):
                if info["kind"] != "shard":
                    continue
                base, ncols = info["base"], info["ncols"]
                nch = K * ncols
                ns, nmax = info["ns"], info["nmax"]
                soff, moff = info["soff"], info["moff"]
                is_last_shard = info is wave_info[len(shard_waves) - 1] or \
                    info.get("extras") is not None

                if info["view0"]:
                    hch = h_bf
                    hch_off = 0
                    ccg_ap = c_sl[:, 0:nch]
                else:
                    hcht = wk.tile([P, KC, NCHMX], BF16, tag="hch")
                    ccg = wk.tile([P, NCHMX], F32, tag="ccg")
                    if info["has_missing"]:
                        nc.vector.memset(hcht[:, :, :nch], 0.0)
                        nc.vector.memset(ccg[:, :nch], 0.0)
                    for ri, (dst, src, ln) in enumerate(info["runs"]):
                        eng = nc.vector if ri % 2 == 0 else nc.scalar
                        if ri % 2 == 0:
                            eng.tensor_copy(hcht[:, :, dst:dst + ln],
                                            h_bf[:, :, src:src + ln])
                        else:
                            eng.copy(hcht[:, :, dst:dst + ln],
                                     h_bf[:, :, src:src + ln])
                    for ri, (dst, src, ln) in enumerate(info["runs"]):
                        if ri % 2 == 0:
                            nc.scalar.copy(ccg[:, dst:dst + ln],
                                           c_sl[:, src:src + ln])
                        else:
                            nc.vector.tensor_copy(ccg[:, dst:dst + ln],
                                                  c_sl[:, src:src + ln])
                    hch = hcht
                    hch_off = 0
                    ccg_ap = ccg[:, :nch]

                hsum_f = wk.tile([P, KC, NMAXC], F32, tag="hsumf")
                nc.vector.tensor_reduce(
                    hsum_f[:, :, :ncols],
                    hch[:, :, hch_off:hch_off + nch].rearrange(
                        "p k (n c) -> p k n c", c=K),
                    axis=mybir.AxisListType.X, op=mybir.AluOpType.add)
                hsum_b = wk.tile([P, KC, NMAXC], BF16, tag="hsumb")
                nc.vector.tensor_copy(hsum_b[:, :, :ncols],
                                      hsum_f[:, :, :ncols])

                psr = pp.tile([P, KC, P], F32, tag="psr")
                msels = []
                for s in range(ns):
                    msel = wk.tile([P, KC, NMAXC], BF16, tag="msel" + str(s))
                    mo = moff + s * KC * ncols
                    nc.vector.tensor_mul(
                        msel[:, :, :ncols], hsum_b[:, :, :ncols],
                        mbuf[:, mo:mo + KC * ncols].rearrange(
                            "p (k n) -> p k n", k=KC))
                    msels.append(msel)
                for m in range(KC):
                    for s in range(ns):
                        wst = wtile[soff + s]
                        for k in range(KC):
                            nc.tensor.matmul(
                                psr[:, m, :ncols],
                                wst[:, m * KC + k, :],
                                msels[s][:, k, :ncols],
                                start=(s == 0 and k == 0),
                                stop=(s == ns - 1 and k == KC - 1))
                tg = str(wi)
                csp = wk.tile([P, KC * nmax], F32, tag="csp" + tg)
                nc.vector.tensor_reduce(
                    csp[:].rearrange("p (k t) -> p k t", k=KC),
                    psr[:, :, :ncols].rearrange("p k (g t) -> p k t g",
                                                t=nmax),
                    axis=mybir.AxisListType.X, op=mybir.AluOpType.add)
                csb = wk.tile([P, KC * nmax], BF16, tag="csb" + tg)
                nc.scalar.activation(csb[:], csp[:], ACT.Identity)
                gin = dp.tile([P, KC, nmax], BF16, tag="gcs_in" + tg)
                nc.sync.dma_start(
                    gin[:], csb[:].rearrange("p (k t) -> p k t", k=KC))
                gout = dp.tile([NCORES, P, KC, nmax], BF16,
                               tag="gcs_out" + tg, addr_space="Shared")
                nc.gpsimd.collective_compute(
                    "AllGather", mybir.AluOpType.bypass,
                    ins=[gin.opt()], outs=[gout.opt()],
                    replica_groups=[list(range(NCORES))])
                chs = wk.tile([P, KC, NMAXC], BF16, tag="chs")
                for g in range(NCORES):
                    eng = nc.sync if g % 2 == 0 else nc.scalar
                    eng.dma_start(chs[:, :, g * nmax:(g + 1) * nmax], gout[g])

                # fh matmuls overlap the AG flight (independent of ch_sum)
                psf = pg.tile([P, NCHMX], F32, tag="psf")
                for k in range(KC):
                    nc.tensor.matmul(
                        psf[:, :nch], fhs[:, k, :],
                        hch[:, k, hch_off:hch_off + nch],
                        start=(k == 0), stop=(k == KC - 1))

                psi = pg.tile([P, 3, P], F32, tag="psi")
                for g in range(3):
                    for k in range(KC):
                        nc.tensor.matmul(
                            psi[:, g, :ncols], iouhs[:, k * 3 + g, :],
                            chs[:, k, :ncols],
                            start=(k == 0), stop=(k == KC - 1))

                n = ncols
                tmp = wk.tile([P, 3, NMAXC], F32, tag="gtmp")
                nc.vector.tensor_add(tmp[:, :, :n], psi[:, :, :n],
                                     xi_f[:, :, base:base + n])
                ig = wk.tile([P, NMAXC], F32, tag="ig")
                og = wk.tile([P, NMAXC], F32, tag="og")
                ug = wk.tile([P, NMAXC], F32, tag="ug")
                nc.scalar.activation(ig[:, :n], tmp[:, 0, :n], ACT.Sigmoid,
                                     bias=biou[:, 0:1])
                nc.scalar.activation(og[:, :n], tmp[:, 1, :n], ACT.Sigmoid,
                                     bias=biou[:, 1:2])
                nc.scalar.activation(ug[:, :n], tmp[:, 2, :n], ACT.Tanh,
                                     bias=biou[:, 2:3])
                cn = wk.tile([P, NMAXC], F32, tag="cn")
                nc.vector.tensor_mul(cn[:, :n], ig[:, :n], ug[:, :n])
                fsb = wk.tile([P, NCHMX], F32, tag="fsb")
                xfb = wk.tile([P, NCHMX], F32, tag="xfb")
                xfb_v = xfb[:, :nch].rearrange("p (n k) -> p n k", k=K)
                for kk in range(K):
                    nc.vector.tensor_copy(
                        xfb_v[:, :, kk:kk + 1],
                        xf_f[:, base:base + n].rearrange(
                            "p (n one) -> p n one", one=1))
                nc.vector.tensor_add(fsb[:, :nch], psf[:, :nch], xfb[:, :nch])
                nc.scalar.activation(fsb[:, :nch], fsb[:, :nch],
                                     ACT.Sigmoid, bias=bfh[:, 0:1])
                nc.vector.tensor_mul(fsb[:, :nch], fsb[:, :nch], ccg_ap)
                fc = wk.tile([P, NMAXC], F32, tag="fc")
                nc.vector.tensor_reduce(
                    fc[:, :n],
                    fsb[:, :nch].rearrange("p (n k) -> p n k", k=K),
                    axis=mybir.AxisListType.X, op=mybir.AluOpType.add)
                nc.vector.tensor_add(cn[:, :n], cn[:, :n], fc[:, :n])
                nc.vector.tensor_copy(c_sl[:, base:base + n], cn[:, :n])
                tct = wk.tile([P, NMAXC], F32, tag="tct")
                nc.scalar.activation(tct[:, :n], cn[:, :n], ACT.Tanh)
                nc.vector.tensor_mul(h_sl[:, base:base + n], og[:, :n],
                                     tct[:, :n])
                publish(base, ncols, str(wi + 1),
                        c_extra=info.get("extras"))

            # ---- replicated-full top waves ---------------------------------
            for info in wave_info:
                if info["kind"] != "replf":
                    continue
                base, ncols = info["base"], info["ncols"]
                nch = K * ncols
                foff, cpos0 = info["foff"], info["cpos0"]

                hchF = wk.tile([P, KC, K * REPLF_MAX], BF16, tag="hchF")
                ccgF = wk.tile([P, KC, K * REPLF_MAX], BF16, tag="ccgF")
                if info["has_missing"]:
                    nc.vector.memset(hchF[:, :, :nch], 0.0)
                    nc.vector.memset(ccgF[:, :, :nch], 0.0)
                for (dst, src, ln) in info["runs"]:
                    nc.vector.tensor_copy(hchF[:, :, dst:dst + ln],
                                          h_bf[:, :, src:src + ln])
                for (dst, src, ln) in info["cruns"]:
                    nc.vector.tensor_copy(ccgF[:, :, dst:dst + ln],
                                          cfull[:, :, src:src + ln])
                hsF = wk.tile([P, KC, REPLF_MAX], F32, tag="hsF")
                nc.vector.tensor_reduce(
                    hsF[:, :, :ncols],
                    hchF[:, :, :nch].rearrange("p k (n c) -> p k n c", c=K),
                    axis=mybir.AxisListType.X, op=mybir.AluOpType.add)
                hsFb = wk.tile([P, KC, REPLF_MAX], BF16, tag="hsFb")
                nc.vector.tensor_copy(hsFb[:, :, :ncols], hsF[:, :, :ncols])

                # replicated relation matmuls: per-rel contiguous col ranges
                psR = pp.tile([P, KC, P], F32, tag="psr")
                for m in range(KC):
                    for si, (r, lo, hi) in enumerate(info["ranges"]):
                        wst = wtile[info["soff"] + si]
                        for k in range(KC):
                            nc.tensor.matmul(
                                psR[:, m, lo:hi], wst[:, m * KC + k, :],
                                hsFb[:, k, lo:hi],
                                start=(k == 0), stop=(k == KC - 1))
                chsF = wk.tile([P, KC, REPLF_MAX], BF16, tag="chsF")
                for k in range(KC):
                    nc.scalar.activation(chsF[:, k, :ncols],
                                         psR[:, k, :ncols], ACT.Identity,
                                         scale=1.0 / (WSCALE * WSCALE))

                # full fh (x WSCALE)
                psfF = pg.tile([P, KC, K * REPLF_MAX], F32, tag="psfF")
                for m in range(KC):
                    for k in range(KC):
                        nc.tensor.matmul(
                            psfF[:, m, :nch], fhFt[:, m * KC + k, :],
                            hchF[:, k, :nch],
                            start=(k == 0), stop=(k == KC - 1))

                # full iou
                psiF = pg.tile([P, 3 * KC, REPLF_MAX], F32, tag="psiF")
                for mg in range(3 * KC):
                    for k in range(KC):
                        nc.tensor.matmul(
                            psiF[:, mg, :ncols], iouhFt[:, mg * KC + k, :],
                            chsF[:, k, :ncols],
                            start=(k == 0), stop=(k == KC - 1))

                tmpF = wk.tile([P, 3 * KC, REPLF_MAX], F32, tag="tmpF")
                nc.vector.tensor_add(tmpF[:, :, :ncols], psiF[:, :, :ncols],
                                     xiFt[:, :, foff:foff + ncols])
                igF = wk.tile([P, KC, REPLF_MAX], F32, tag="igF")
                ogF = wk.tile([P, KC, REPLF_MAX], F32, tag="ogF")
                ugF = wk.tile([P, KC, REPLF_MAX], F32, tag="ugF")
                for m in range(KC):
                    nc.scalar.activation(igF[:, m, :ncols],
                                         tmpF[:, m, :ncols], ACT.Sigmoid)
                    nc.scalar.activation(ogF[:, m, :ncols],
                                         tmpF[:, KC + m, :ncols], ACT.Sigmoid)
                    nc.scalar.activation(ugF[:, m, :ncols],
                                         tmpF[:, 2 * KC + m, :ncols], ACT.Tanh)
                cnF = wk.tile([P, KC, REPLF_MAX], F32, tag="cnF")
                nc.vector.tensor_mul(cnF[:, :, :ncols], igF[:, :, :ncols],
                                     ugF[:, :, :ncols])
                # f = sigmoid((psfF + WSCALE*(xf+b)) / WSCALE)
                fsbF = wk.tile([P, KC, K * REPLF_MAX], F32, tag="fsbF")
                xfbF = wk.tile([P, KC, K * REPLF_MAX], F32, tag="xfbF")
                for m in range(KC):
                    xfb_v = xfbF[:, m, :nch].rearrange("p (n k) -> p n k", k=K)
                    for kk in range(K):
                        nc.vector.tensor_copy(
                            xfb_v[:, :, kk:kk + 1],
                            xffFt[:, m, foff:foff + ncols].rearrange(
                                "p (n one) -> p n one", one=1))
                nc.vector.tensor_add(fsbF[:, :, :nch], psfF[:, :, :nch],
                                     xfbF[:, :, :nch])
                for m in range(KC):
                    nc.scalar.activation(fsbF[:, m, :nch], fsbF[:, m, :nch],
                                         ACT.Sigmoid, scale=1.0 / WSCALE)
                nc.vector.tensor_mul(fsbF[:, :, :nch], fsbF[:, :, :nch],
                                     ccgF[:, :, :nch])
                fcF = wk.tile([P, KC, REPLF_MAX], F32, tag="fcF")
                nc.vector.tensor_reduce(
                    fcF[:, :, :ncols],
                    fsbF[:, :, :nch].rearrange("p m (n k) -> p m n k", k=K),
                    axis=mybir.AxisListType.X, op=mybir.AluOpType.add)
                nc.vector.tensor_add(cnF[:, :, :ncols], cnF[:, :, :ncols],
                                     fcF[:, :, :ncols])
                thF = wk.tile([P, KC, REPLF_MAX], F32, tag="thF")
                for m in range(KC):
                    nc.scalar.activation(thF[:, m, :ncols],
                                         cnF[:, m, :ncols], ACT.Tanh)
                hF = wk.tile([P, KC, REPLF_MAX], F32, tag="hF")
                nc.vector.tensor_mul(hF[:, :, :ncols], ogF[:, :, :ncols],
                                     thF[:, :, :ncols])
                # update replicated state
                nc.vector.tensor_copy(h_bf[:, :, base:base + ncols],
                                      hF[:, :, :ncols])
                nc.vector.tensor_copy(cfull[:, :, cpos0:cpos0 + ncols],
                                      cnF[:, :, :ncols])
                # extract own feature slice via select mask
                sh = wk.tile([P, KC, REPLF_MAX], F32, tag="selh")
                nc.vector.tensor_mul(sh[:, :, :ncols], hF[:, :, :ncols],
                                     selmt[:, :, foff:foff + ncols])
                nc.vector.tensor_reduce(
                    h_sl[:, base:base + ncols],
                    sh[:, :, :ncols].rearrange("p k n -> p n k"),
                    axis=mybir.AxisListType.X, op=mybir.AluOpType.add)

            nc.sync.dma_start(d_hout.ap(), h_sl[:])

    in_maps = []
    for c in range(NCORES):
        in_maps.append({
            "wstream": wstream[c], "masks": maskbuf[c],
            "xt": xT_b, "iouxstat": iouxstat[c], "iouhstat": iouhstat[c],
            "fxstat": fxstat[c], "fhstat": fhstat[c],
            "iouhF": iouhF, "fhF": fhF, "xiF": xiF, "xffF": xffF,
            "selm": selm[c],
            "b_xi": b_xi[c], "b_iou": b_iou[c], "b_xf": b_xf[c],
            "b_fh": b_fh[c],
        })
    _split_multi_waits(nc)
    kernel._nc = nc
    kernel._in_maps = in_maps
    res = run_bass_kernel_spmd(nc, in_maps, list(range(NCORES)))
    hT = np.concatenate([res.results[c]["hout"] for c in range(NCORES)], 0)
    out = np.empty((N, MEM), np.float32)
    for node in range(N):
        out[node] = hT[:, col_of[node]]
    return out


# revision 38
# speedup vs baseline: 1.1685x; 1.1685x over previous
"""ChildSumTreeLSTM with relation transforms on 8 Trainium2 NeuronCores.

Layout: transposed (features on SBUF partitions, tree nodes on the free dim),
feature-sharded state (each core owns a 128-feature slice of h/c/xi/gates).

Column order: the leaf region is laid out in wave-1 child-slot order
(child k of wave-1 grouped parent t sits at column 4t+k, missing/pad slots
are dummy zero columns) so wave-1's child gather is a plain view.  Each
sharded internal wave's parent columns are grouped by the core that owns
their relation (8 equal padded groups) so every exchange is an AllGather
of the core's own contiguous shard — no AllReduce anywhere:

  per sharded wave: hsum -> per-slot (rel) masked-rhs matmuls accumulated
  into one PSUM region (mask selects that rel's columns; the per-core psum
  is then nonzero only on the core's own column group) -> group-axis
  reduce packs the AG shard -> AllGather ch_sum -> column-sharded iou
  gates + f gates -> AllGather of the new h feature-slices.

The tiny top waves (3, 4, 5 — 8 nodes) run fully REPLICATED on every core
(full-feature compute, relation weights replicated, host-precomputed xi),
eliminating their collectives; each core extracts its own feature slice of
the results with a data-driven select mask.  AGh2 additionally publishes
the c state those waves need.  Relation weights are fp8 (x WSCALE), with
1/WSCALE folded into the iouh weights / activation scales.  A warmup
collective at t~0 absorbs the ~52us collectives-firmware spin-up.
All per-core differences are input data, so one Bass program runs SPMD.
"""

import sys

sys.path.insert(0, "/opt/trn_rl_repo")

import numpy as np
import ml_dtypes

import concourse.bass as bass
import concourse.mybir as mybir
import concourse.tile as tile
from concourse.bass_utils import run_bass_kernel_spmd
from concourse.vector_clock import ScopedClock, VectorClock

BF16 = mybir.dt.bfloat16
FP8 = mybir.dt.float8e4
F32 = mybir.dt.float32
NCORES = 8
P = 128
WSCALE = 64.0   # fp8 weights are stored x WSCALE
REPLF_MAX = 8   # waves with <= this many cols run replicated on all cores


def _split_drain_and_barrier(self, tick_clock, wait_clock):
    gc = tick_clock.global_clock
    n = len(gc)
    nonzero = [i for i in range(n) if gc[i] > 0]
    for j in nonzero:
        vec = VectorClock([gc[i] if i == j else 0 for i in range(n)])
        d = self.nc.sync.drain()
        wait_clock.add_sem_waits(d.ins, ScopedClock({None: vec}))
    if not nonzero:
        d = self.nc.sync.drain()
        wait_clock.add_sem_waits(d.ins, ScopedClock({None: gc.copy()}))
    self.nc.all_engine_barrier()
    assert self.sems is not None
    popped = self.nc._tile_sem_poison_stack.pop()
    assert popped is self._sem_poison
    self.nc.clear_and_free_semaphores(list(self.sems.allocated().values()))
    self.nc.all_engine_barrier()


tile.TileContext._drain_and_barrier = _split_drain_and_barrier


def _split_multi_waits(nc, limit=1):
    for bb in nc.main_func.blocks:
        new_list = []
        for ins in bb.instructions:
            si = getattr(ins, "sync_info", None)
            if si is not None and si.on_wait and len(si.on_wait) > limit:
                waits = list(si.on_wait)
                for w in waits[:-limit]:
                    nop = mybir.InstNoOp(
                        name=nc.get_next_instruction_name(),
                        sync_info=mybir.SyncInfo(on_wait=[w], on_update=[]),
                        bass_nofuse=True,
                        engine=ins.engine,
                    )
                    nc.register_instruction(nop, overwrite=True)
                    new_list.append(nop)
                si.on_wait = waits[-limit:]
            new_list.append(ins)
        bb.instructions[:] = new_list


def _bf16(a):
    return np.ascontiguousarray(a.astype(ml_dtypes.bfloat16))


def _fp8w(a):
    return (np.asarray(a, np.float32) * WSCALE).astype(ml_dtypes.float8_e4m3)


def _blocksT(mat):
    """[M, K] -> [M/128 * K/128, 128, 128]; index m*KC+k holds mat[mb,kb].T"""
    M, K = mat.shape
    MC, KC = M // P, K // P
    out = np.empty((MC * KC, P, P), mat.dtype)
    for m in range(MC):
        for k in range(KC):
            out[m * KC + k] = mat[m * P:(m + 1) * P, k * P:(k + 1) * P].T
    return out


def _runs(seq, zcol):
    runs = []
    i0 = 0
    n = len(seq)
    while i0 < n:
        if seq[i0] == zcol:
            i0 += 1
            continue
        i1 = i0 + 1
        while i1 < n and seq[i1] == seq[i1 - 1] + 1 and seq[i1] != zcol:
            i1 += 1
        runs.append((i0, int(seq[i0]), i1 - i0))
        i0 = i1
    return runs


def _plan(child_idx, rel_ids, Wrel):
    N, K = child_idx.shape
    eff_children = []
    wave = np.zeros(N, np.int32)
    for i in range(N):
        cs = [int(c) for c in child_idx[i] if 0 <= c < i]
        eff_children.append(cs)
        wave[i] = 1 + max((wave[c] for c in cs), default=-1)
    nwaves = int(wave.max()) + 1
    wave_nodes = [sorted([i for i in range(N) if wave[i] == w],
                         key=lambda i: -i) for w in range(nwaves)]

    from collections import Counter
    wave_info = []
    for w in range(1, nwaves):
        nodes = wave_nodes[w]
        if len(nodes) <= REPLF_MAX:
            # replicated-full wave: order cols by (rel, -node); per-rel ranges
            grouped = sorted(nodes, key=lambda i: (int(rel_ids[i]), -i))
            ranges = []  # (rel, lo, hi)
            for t, node in enumerate(grouped):
                r = int(rel_ids[node])
                if ranges and ranges[-1][0] == r:
                    ranges[-1][2] = t + 1
                else:
                    ranges.append([r, t, t + 1])
            wave_info.append(dict(kind="replf", nodes=nodes, grouped=grouped,
                                  ncols=len(grouped), ranges=ranges))
        else:
            cnt = Counter(int(rel_ids[i]) for i in nodes)
            rels = sorted(cnt, key=lambda r: -cnt[r])
            ns = (len(rels) + NCORES - 1) // NCORES
            core_rels = [[] for _ in range(NCORES)]
            core_cols = [0] * NCORES
            for r in rels:
                best = min(range(NCORES),
                           key=lambda c: (len(core_rels[c]) >= ns,
                                          core_cols[c], len(core_rels[c])))
                core_rels[best].append(r)
                core_cols[best] += cnt[r]
            nmax = max(core_cols)
            grouped = []
            for c in range(NCORES):
                cn = [i for i in nodes if int(rel_ids[i]) in core_rels[c]]
                cn.sort(key=lambda i: (int(rel_ids[i]), -i))
                grouped.extend(cn)
                grouped.extend([None] * (nmax - len(cn)))
            wave_info.append(dict(kind="shard", nodes=nodes, ns=ns,
                                  core_rels=core_rels, nmax=nmax,
                                  grouped=grouped, ncols=NCORES * nmax))

    # ---- global column order ------------------------------------------------
    # leaf region: wave-1 child-slot order (4 slots per wave-1 grouped col),
    # then any leaves that are not children of wave-1 parents.
    col_of = np.full(N, -1, np.int64)
    w1 = wave_info[0]
    leafdata = []  # node or None per col
    for node in w1["grouped"]:
        cs = eff_children[node] if node is not None else []
        leafdata.extend(cs + [None] * (K - len(cs)))
    placed = set(c for c in leafdata if c is not None)
    orphans = [i for i in wave_nodes[0] if i not in placed]
    leafdata.extend(orphans)
    for j, node in enumerate(leafdata):
        if node is not None:
            col_of[node] = j
    order_cols = list(leafdata)
    n0 = len(order_cols)
    for info in wave_info:
        info["base"] = len(order_cols)
        for node in info["grouped"]:
            if node is not None:
                col_of[node] = len(order_cols)
            order_cols.append(node)
    C = len(order_cols)
    ZCOL = C
    NPAD = C + 6

    # ---- child gather runs (wave 1 is a pure view of the leaf region) ------
    for wi, info in enumerate(wave_info):
        if wi == 0:
            info["view0"] = True
            info["runs"] = []
            info["has_missing"] = False
            continue
        info["view0"] = False
        seq = []
        for node in info["grouped"]:
            if node is None:
                seq.extend([ZCOL] * K)
            else:
                cs = eff_children[node]
                seq.extend([int(col_of[c]) for c in cs])
                seq.extend([ZCOL] * (K - len(cs)))
        info["runs"] = _runs(seq, ZCOL)
        info["has_missing"] = any(s == ZCOL for s in seq)
        info["child_cols"] = [s for s in seq]

    # ---- c-state publication plan for the replicated waves ------------------
    # cfull positions: [last-shard-wave cols + extras] ++ [replf waves' cols]
    shard_is = [i for i, x in enumerate(wave_info) if x["kind"] == "shard"]
    repl_is = [i for i, x in enumerate(wave_info) if x["kind"] == "replf"]
    lastsh = wave_info[shard_is[-1]]
    repl_colset = set()
    for i in repl_is:
        info = wave_info[i]
        repl_colset.update(range(info["base"], info["base"] + info["ncols"]))
    need_c = set()
    for i in repl_is:
        for s in wave_info[i]["child_cols"]:
            if s != ZCOL and s not in repl_colset:
                need_c.add(s)
    sh_cols = set(range(lastsh["base"], lastsh["base"] + lastsh["ncols"]))
    extras = sorted(need_c - sh_cols)
    pub_cols = list(range(lastsh["base"], lastsh["base"] + lastsh["ncols"])) \
        + extras
    cpos_of = {c: t for t, c in enumerate(pub_cols)}
    t = len(pub_cols)
    for i in repl_is:
        info = wave_info[i]
        info["cpos0"] = t
        for cc in range(info["base"], info["base"] + info["ncols"]):
            cpos_of[cc] = t
            t += 1
    CFN = t
    lastsh["extras"] = extras
    # replf gather runs in cfull coordinates
    for i in repl_is:
        info = wave_info[i]
        cseq = [cpos_of.get(s, -1) if s != ZCOL else -1
                for s in info["child_cols"]]
        assert all(p >= 0 for p, s in zip(cseq, info["child_cols"])
                   if s != ZCOL)
        info["cruns"] = _runs([p if s != ZCOL else CFN
                               for p, s in zip(cseq, info["child_cols"])], CFN)

    return dict(wave_nodes=wave_nodes, wave_info=wave_info, col_of=col_of,
                order_cols=order_cols, C=C, ZCOL=ZCOL, NPAD=NPAD, n0=n0,
                CFN=CFN, eff_children=eff_children)


def kernel(**inputs):
    x = np.asarray(inputs["x"], np.float32)
    Wrel = np.asarray(inputs["Wrel"], np.float32)
    ioux_w = np.asarray(inputs["ioux_w"], np.float32)
    ioux_b = np.asarray(inputs["ioux_b"], np.float32)
    iouh_w = np.asarray(inputs["iouh_w"], np.float32)
    iouh_b = np.asarray(inputs["iouh_b"], np.float32)
    fx_w = np.asarray(inputs["fx_w"], np.float32)
    fx_b = np.asarray(inputs["fx_b"], np.float32)
    fh_w = np.asarray(inputs["fh_w"], np.float32)
    fh_b = np.asarray(inputs["fh_b"], np.float32)
    child_idx = np.asarray(inputs["child_idx"], np.int32)
    rel_ids = np.asarray(inputs["rel_ids"], np.int32)

    # dummy leaf slots rely on zero iou biases producing h == c == 0
    assert np.allclose(ioux_b, 0) and np.allclose(iouh_b, 0)

    N, IN_DIM = x.shape
    MEM = fh_w.shape[0]
    KC = MEM // P
    KX = IN_DIM // P
    K = child_idx.shape[1]

    plan = _plan(child_idx, rel_ids, Wrel)
    wave_info, col_of = plan["wave_info"], plan["col_of"]
    order_cols, C, NPAD = plan["order_cols"], plan["C"], plan["NPAD"]
    n0, CFN = plan["n0"], plan["CFN"]

    # ---- per-core host data -------------------------------------------------
    xT = np.zeros((IN_DIM, C), np.float32)
    for j, node in enumerate(order_cols):
        if node is not None:
            xT[:, j] = x[node]
    # partition-major host layout -> contiguous DMA (no descriptor flood)
    xT_b = np.zeros((P, KX, C), ml_dtypes.bfloat16)
    for k in range(KX):
        xT_b[:, k, :] = _bf16(xT[k * P:(k + 1) * P])

    shard_waves = [i for i in wave_info if i["kind"] == "shard"]
    repl_waves = [i for i in wave_info if i["kind"] == "replf"]
    S_total = sum(i["ns"] for i in shard_waves) \
        + sum(len(i["ranges"]) for i in repl_waves)
    MTK = sum(i["ns"] * KC * i["ncols"] for i in shard_waves)

    wstream = [np.zeros((S_total, P, KC * KC, P), ml_dtypes.float8_e4m3)
               for _ in range(NCORES)]
    maskbuf = [np.zeros((P, max(MTK, 1)), ml_dtypes.bfloat16)
               for _ in range(NCORES)]
    eyeW = np.eye(MEM, dtype=np.float32)
    soff = 0
    moff = 0
    for info in wave_info:
        if info["kind"] == "shard":
            ncols = info["ncols"]
            info["soff"], info["moff"] = soff, moff
            for c in range(NCORES):
                for s, r in enumerate(info["core_rels"][c]):
                    wstream[c][soff + s] = _fp8w(
                        _blocksT(Wrel[r]).transpose(1, 0, 2))
                    m = np.zeros((KC, ncols), np.float32)
                    for t in range(ncols):
                        node = info["grouped"][t]
                        if node is not None and int(rel_ids[node]) == r:
                            m[:, t] = 1.0
                    mo = moff + s * KC * ncols
                    maskbuf[c][:, mo:mo + KC * ncols] = _bf16(
                        np.broadcast_to(m.reshape(1, -1), (P, KC * ncols)))
            soff += info["ns"]
            moff += info["ns"] * KC * ncols
        else:
            info["soff"] = soff
            for (r, lo, hi) in info["ranges"]:
                Wm = Wrel[r] if r < Wrel.shape[0] else eyeW
                wb = _fp8w(_blocksT(Wm).transpose(1, 0, 2))
                for c in range(NCORES):
                    wstream[c][soff] = wb
                soff += 1

    # full-feature constants for the replicated waves (fp8 x WSCALE);
    # block order: [g*KC + mchunk] for iouh, [m*KC + k] plain for fh
    iouhF = np.zeros((3 * KC * KC, P, P), ml_dtypes.float8_e4m3)
    bT = _blocksT(iouh_w)  # [(3KC)*KC]
    for mg in range(3 * KC):
        for k in range(KC):
            iouhF[mg * KC + k] = _fp8w(bT[mg * KC + k])
    fhF = np.zeros((KC * KC, P, P), ml_dtypes.float8_e4m3)
    bT = _blocksT(fh_w)
    for i in range(KC * KC):
        fhF[i] = _fp8w(bT[i])
    # host xi / xf for replicated cols (true scale; biases folded in)
    NF = sum(i["ncols"] for i in repl_waves)
    xiF = np.zeros((3 * KC, P, max(NF, 1)), np.float32)
    xffF = np.zeros((KC, P, max(NF, 1)), np.float32)
    selm = [np.zeros((P, KC, max(NF, 1)), np.float32) for _ in range(NCORES)]
    off = 0
    for info in repl_waves:
        info["foff"] = off
        for t, node in enumerate(info["grouped"]):
            xi_full = ioux_w @ x[node] + ioux_b + iouh_b  # [3*MEM]
            xf_full = (fx_w @ x[node] + fx_b + fh_b) * WSCALE
            for g in range(3):
                for m in range(KC):
                    xiF[g * KC + m, :, off + t] = \
                        xi_full[g * MEM + m * P:(g * MEM + (m + 1) * P)]
            for m in range(KC):
                xffF[m, :, off + t] = xf_full[m * P:(m + 1) * P]
        off += info["ncols"]
    for c in range(NCORES):
        selm[c][:, c, :] = 1.0

    iouxstat = [np.zeros((KX * 3, P, P), ml_dtypes.bfloat16)
                for _ in range(NCORES)]
    iouhstat = [np.zeros((KC * 3, P, P), ml_dtypes.bfloat16)
                for _ in range(NCORES)]
    fxstat = [np.zeros((KX, P, P), ml_dtypes.bfloat16) for _ in range(NCORES)]
    fhstat = [np.zeros((KC, P, P), ml_dtypes.bfloat16) for _ in range(NCORES)]
    b_xi = [np.zeros((3, P), np.float32) for _ in range(NCORES)]
    b_iou = [np.zeros((3, P), np.float32) for _ in range(NCORES)]
    b_xf = [np.zeros((P,), np.float32) for _ in range(NCORES)]
    b_fh = [np.zeros((P,), np.float32) for _ in range(NCORES)]
    for c in range(NCORES):
        rows = slice(c * P, (c + 1) * P)
        for g in range(3):
            gr = slice(g * MEM + c * P, g * MEM + (c + 1) * P)
            b_xi[c][g] = ioux_b[gr]
            b_iou[c][g] = iouh_b[gr]
            for k in range(KX):
                iouxstat[c][k * 3 + g] = _bf16(ioux_w[gr, k * P:(k + 1) * P].T)
            for k in range(KC):
                iouhstat[c][k * 3 + g] = _bf16(
                    iouh_w[gr, k * P:(k + 1) * P].T / WSCALE)
        b_xf[c] = fx_b[rows]
        b_fh[c] = fh_b[rows]
        for k in range(KX):
            fxstat[c][k] = _bf16(fx_w[rows, k * P:(k + 1) * P].T)
        for k in range(KC):
            fhstat[c][k] = _bf16(fh_w[rows, k * P:(k + 1) * P].T)

    # ---- build program ------------------------------------------------------
    nc = bass.Bass("TRN2", target_bir_lowering=False, debug=False,
                   num_devices=NCORES)
    d_ws = nc.dram_tensor("wstream", list(wstream[0].shape), FP8,
                          kind="ExternalInput")
    d_mask = nc.dram_tensor("masks", list(maskbuf[0].shape), BF16,
                            kind="ExternalInput")
    d_xt = nc.dram_tensor("xt", [P, KX, C], BF16, kind="ExternalInput")
    d_iouxs = nc.dram_tensor("iouxstat", [P, KX * 3, P], BF16,
                             kind="ExternalInput")
    d_iouhs = nc.dram_tensor("iouhstat", [P, KC * 3, P], BF16,
                             kind="ExternalInput")
    d_fxs = nc.dram_tensor("fxstat", [P, KX, P], BF16, kind="ExternalInput")
    d_fhs = nc.dram_tensor("fhstat", [P, KC, P], BF16, kind="ExternalInput")
    d_iouhF = nc.dram_tensor("iouhF", [P, 3 * KC * KC, P], FP8,
                             kind="ExternalInput")
    d_fhF = nc.dram_tensor("fhF", [P, KC * KC, P], FP8, kind="ExternalInput")
    d_xiF = nc.dram_tensor("xiF", [P, 3 * KC, max(NF, 1)], F32,
                           kind="ExternalInput")
    d_xffF = nc.dram_tensor("xffF", [P, KC, max(NF, 1)], F32,
                            kind="ExternalInput")
    d_selm = nc.dram_tensor("selm", [P, KC, max(NF, 1)], F32,
                            kind="ExternalInput")
    d_bxi = nc.dram_tensor("b_xi", [3, P], F32, kind="ExternalInput")
    d_biou = nc.dram_tensor("b_iou", [3, P], F32, kind="ExternalInput")
    d_bxf = nc.dram_tensor("b_xf", [P], F32, kind="ExternalInput")
    d_bfh = nc.dram_tensor("b_fh", [P], F32, kind="ExternalInput")
    d_hout = nc.dram_tensor("hout", [P, C], F32, kind="ExternalOutput")

    ACT = mybir.ActivationFunctionType
    NMAXC = max([i["ncols"] for i in shard_waves] + [1])
    NCHMX = K * NMAXC
    NFC = max(NF, 1)

    with tile.TileContext(nc, num_cores=NCORES) as tc:
        with (
            tc.tile_pool(name="const", bufs=1) as cpool,
            tc.tile_pool(name="state", bufs=1) as spool,
            tc.tile_pool(name="wstage", bufs=8) as wpool,
            tc.tile_pool(name="work", bufs=1) as wk,
            tc.tile_pool(name="psum", bufs=1, space="PSUM") as pp,
            tc.tile_pool(name="psg", bufs=1, space="PSUM") as pg,
            tc.tile_pool(name="dram", bufs=2, space="DRAM") as dp,
        ):
            # ---- warmup collective: absorbs the CC firmware spin-up --------
            wu = cpool.tile([P, 8], BF16)
            nc.vector.memset(wu[:], 0.0)
            wu_in = dp.tile([P, 8], BF16, tag="wu_in")
            nc.scalar.dma_start(wu_in[:], wu[:])
            wu_out = dp.tile([NCORES, P, 8], BF16, tag="wu_out",
                             addr_space="Shared")
            nc.gpsimd.collective_compute(
                "AllGather", mybir.AluOpType.bypass,
                ins=[wu_in.opt()], outs=[wu_out.opt()],
                replica_groups=[list(range(NCORES))])

            # ---- constants (SP queue; all contiguous partition-major) ------
            xt = cpool.tile([P, KX, C], BF16)
            nc.sync.dma_start(xt[:], d_xt.ap())
            iouxs = cpool.tile([P, KX * 3, P], BF16)
            nc.sync.dma_start(iouxs[:], d_iouxs.ap())
            iouhs = cpool.tile([P, KC * 3, P], BF16)
            nc.sync.dma_start(iouhs[:], d_iouhs.ap())
            fxs = cpool.tile([P, KX, P], BF16)
            nc.sync.dma_start(fxs[:], d_fxs.ap())
            fhs = cpool.tile([P, KC, P], BF16)
            nc.sync.dma_start(fhs[:], d_fhs.ap())
            bxi = cpool.tile([P, 3], F32)
            nc.sync.dma_start(bxi[:], d_bxi.ap().rearrange("g p -> p g"))
            biou = cpool.tile([P, 3], F32)
            nc.sync.dma_start(biou[:], d_biou.ap().rearrange("g p -> p g"))
            bxf = cpool.tile([P, 1], F32)
            nc.sync.dma_start(bxf[:], d_bxf.ap().rearrange(
                "(p one) -> p one", one=1))
            bfh = cpool.tile([P, 1], F32)
            nc.sync.dma_start(bfh[:], d_bfh.ap().rearrange(
                "(p one) -> p one", one=1))
            mbuf = cpool.tile([P, max(MTK, 1)], BF16)
            nc.sync.dma_start(mbuf[:], d_mask.ap())
            xiFt = cpool.tile([P, 3 * KC, NFC], F32)
            nc.sync.dma_start(xiFt[:], d_xiF.ap())
            xffFt = cpool.tile([P, KC, NFC], F32)
            nc.sync.dma_start(xffFt[:], d_xffF.ap())
            selmt = cpool.tile([P, KC, NFC], F32)
            nc.sync.dma_start(selmt[:], d_selm.ap())

            # ---- bulk weights (ACT queue; never blocks the SP queue) -------
            wtile = []
            for s in range(S_total):
                t = wpool.tile([P, KC * KC, P], FP8, tag="wst")
                nc.scalar.dma_start(t[:], d_ws.ap()[s])
                wtile.append(t)
            iouhFt = cpool.tile([P, 3 * KC * KC, P], FP8)
            nc.scalar.dma_start(iouhFt[:], d_iouhF.ap())
            fhFt = cpool.tile([P, KC * KC, P], FP8)
            nc.scalar.dma_start(fhFt[:], d_fhF.ap())

            # ---- state -----------------------------------------------------
            h_bf = spool.tile([P, KC, NPAD], BF16)
            nc.vector.memset(h_bf[:], 0.0)
            c_sl = spool.tile([P, NPAD], F32)
            nc.vector.memset(c_sl[:], 0.0)
            h_sl = spool.tile([P, C], F32)
            xi_f = spool.tile([P, 3, C], F32)
            xf_f = spool.tile([P, C], F32)
            cfull = spool.tile([P, KC, CFN + 2], BF16)

            # ---- PE clock warmup: keep the array busy through the initial
            # DMA window so xi runs at full pstate ----------------------------
            pswarm = pg.tile([P, 3, 128], F32, tag="ps3")
            for _ in range(100):
                nc.tensor.matmul(pswarm[:8, 0, :8], wu[:, :8], wu[:, :8],
                                 start=True, stop=True)

            # ---- xi/xf precompute (feature-sharded) ------------------------
            CCH = 128
            for cc in range(0, C, CCH):
                ncc = min(CCH, C - cc)
                ps = pg.tile([P, 3, CCH], F32, tag="ps3")
                for g in range(3):
                    for k in range(KX):
                        nc.tensor.matmul(
                            ps[:, g, :ncc], iouxs[:, k * 3 + g, :],
                            xt[:, k, cc:cc + ncc],
                            start=(k == 0), stop=(k == KX - 1))
                for g in range(3):
                    nc.scalar.activation(
                        xi_f[:, g, cc:cc + ncc], ps[:, g, :ncc],
                        ACT.Identity, bias=bxi[:, g:g + 1])
                psf0 = pg.tile([P, CCH], F32, tag="psf0")
                for k in range(KX):
                    nc.tensor.matmul(
                        psf0[:, :ncc], fxs[:, k, :], xt[:, k, cc:cc + ncc],
                        start=(k == 0), stop=(k == KX - 1))
                nc.scalar.activation(
                    xf_f[:, cc:cc + ncc], psf0[:, :ncc],
                    ACT.Identity, bias=bxf[:, 0:1])

            def publish(p0, ncols, tag, c_extra=None):
                """AllGather this core's h feature-slice for cols
                [p0, p0+ncols); optionally also c for those cols plus the
                c_extra col list (writes cfull[0:. ] in publish order)."""
                rowdim = 2 if c_extra is not None else 1
                nce = ncols + (len(c_extra) if c_extra else 0)
                hb = wk.tile([P, rowdim, nce], BF16, tag="hb" + tag)
                nc.scalar.copy(hb[:, 0, :ncols], h_sl[:, p0:p0 + ncols])
                if c_extra is not None:
                    nc.scalar.copy(hb[:, 1, :ncols], c_sl[:, p0:p0 + ncols])
                    for j, ecol in enumerate(c_extra):
                        nc.scalar.copy(hb[:, 0, ncols + j:ncols + j + 1],
                                       h_sl[:, ecol:ecol + 1])
                        nc.scalar.copy(hb[:, 1, ncols + j:ncols + j + 1],
                                       c_sl[:, ecol:ecol + 1])
                gin = dp.tile([P, rowdim, nce], BF16, tag="gin" + tag)
                nc.sync.dma_start(gin[:], hb[:])
                gout = dp.tile([NCORES, P, rowdim, nce], BF16,
                               tag="gout" + tag, addr_space="Shared")
                nc.gpsimd.collective_compute(
                    "AllGather", mybir.AluOpType.bypass,
                    ins=[gin.opt()], outs=[gout.opt()],
                    replica_groups=[list(range(NCORES))])
                nc.sync.dma_start(
                    h_bf[:, :, p0:p0 + ncols],
                    gout[:, :, 0, :ncols].rearrange("k p n -> p k n"))
                if c_extra is not None:
                    nc.sync.dma_start(
                        cfull[:, :, 0:nce],
                        gout[:, :, 1, :].rearrange("k p n -> p k n"))

            # ---- wave 0: leaves (iou = xi, no children) --------------------
            for cc in range(0, n0, NMAXC):
                L = min(NMAXC, n0 - cc)
                ig = wk.tile([P, NMAXC], F32, tag="ig")
                og = wk.tile([P, NMAXC], F32, tag="og")
                ug = wk.tile([P, NMAXC], F32, tag="ug")
                nc.scalar.activation(ig[:, :L], xi_f[:, 0, cc:cc + L],
                                     ACT.Sigmoid, bias=biou[:, 0:1])
                nc.scalar.activation(og[:, :L], xi_f[:, 1, cc:cc + L],
                                     ACT.Sigmoid, bias=biou[:, 1:2])
                nc.scalar.activation(ug[:, :L], xi_f[:, 2, cc:cc + L],
                                     ACT.Tanh, bias=biou[:, 2:3])
                nc.vector.tensor_mul(c_sl[:, cc:cc + L], ig[:, :L], ug[:, :L])
                tct = wk.tile([P, NMAXC], F32, tag="tct")
                nc.scalar.activation(tct[:, :L], c_sl[:, cc:cc + L], ACT.Tanh)
                nc.vector.tensor_mul(h_sl[:, cc:cc + L], og[:, :L], tct[:, :L])
            publish(0, n0, "0")

            # ---- sharded internal waves ------------------------------------
            for wi, info in enumerate(wave_info):
                if info["kind"] != "shard":
                    continue
                base, ncols = info["base"], info["ncols"]
                nch = K * ncols
                ns, nmax = info["ns"], info["nmax"]
                soff, moff = info["soff"], info["moff"]
                is_last_shard = info is wave_info[len(shard_waves) - 1] or \
                    info.get("extras") is not None

                if info["view0"]:
                    hch = h_bf
                    hch_off = 0
                    ccg_ap = c_sl[:, 0:nch]
                else:
                    hcht = wk.tile([P, KC, NCHMX], BF16, tag="hch")
                    ccg = wk.tile([P, NCHMX], F32, tag="ccg")
                    if info["has_missing"]:
                        nc.vector.memset(hcht[:, :, :nch], 0.0)
                        nc.vector.memset(ccg[:, :nch], 0.0)
                    # c gather first: depends only on c_sl, runs during the
                    # previous publish's AG flight
                    for ri, (dst, src, ln) in enumerate(info["runs"]):
                        if ri % 2 == 0:
                            nc.scalar.copy(ccg[:, dst:dst + ln],
                                           c_sl[:, src:src + ln])
                        else:
                            nc.vector.tensor_copy(ccg[:, dst:dst + ln],
                                                  c_sl[:, src:src + ln])
                    for ri, (dst, src, ln) in enumerate(info["runs"]):
                        if ri % 2 == 0:
                            nc.vector.tensor_copy(hcht[:, :, dst:dst + ln],
                                                  h_bf[:, :, src:src + ln])
                        else:
                            nc.scalar.copy(hcht[:, :, dst:dst + ln],
                                           h_bf[:, :, src:src + ln])
                    hch = hcht
                    hch_off = 0
                    ccg_ap = ccg[:, :nch]

                hsum_f = wk.tile([P, KC, NMAXC], F32, tag="hsumf")
                nc.vector.tensor_reduce(
                    hsum_f[:, :, :ncols],
                    hch[:, :, hch_off:hch_off + nch].rearrange(
                        "p k (n c) -> p k n c", c=K),
                    axis=mybir.AxisListType.X, op=mybir.AluOpType.add)
                hsum_b = wk.tile([P, KC, NMAXC], BF16, tag="hsumb")
                nc.vector.tensor_copy(hsum_b[:, :, :ncols],
                                      hsum_f[:, :, :ncols])

                psr = pp.tile([P, KC, P], F32, tag="psr")
                msels = []
                for s in range(ns):
                    msel = wk.tile([P, KC, NMAXC], BF16, tag="msel" + str(s))
                    mo = moff + s * KC * ncols
                    nc.vector.tensor_mul(
                        msel[:, :, :ncols], hsum_b[:, :, :ncols],
                        mbuf[:, mo:mo + KC * ncols].rearrange(
                            "p (k n) -> p k n", k=KC))
                    msels.append(msel)
                for m in range(KC):
                    for s in range(ns):
                        wst = wtile[soff + s]
                        for k in range(KC):
                            nc.tensor.matmul(
                                psr[:, m, :ncols],
                                wst[:, m * KC + k, :],
                                msels[s][:, k, :ncols],
                                start=(s == 0 and k == 0),
                                stop=(s == ns - 1 and k == KC - 1))
                tg = str(wi)
                csp = wk.tile([P, KC * nmax], F32, tag="csp" + tg)
                nc.vector.tensor_reduce(
                    csp[:].rearrange("p (k t) -> p k t", k=KC),
                    psr[:, :, :ncols].rearrange("p k (g t) -> p k t g",
                                                t=nmax),
                    axis=mybir.AxisListType.X, op=mybir.AluOpType.add)
                csb = wk.tile([P, KC * nmax], BF16, tag="csb" + tg)
                nc.vector.tensor_copy(csb[:], csp[:])
                gin = dp.tile([P, KC, nmax], BF16, tag="gcs_in" + tg)
                nc.sync.dma_start(
                    gin[:], csb[:].rearrange("p (k t) -> p k t", k=KC))
                gout = dp.tile([NCORES, P, KC, nmax], BF16,
                               tag="gcs_out" + tg, addr_space="Shared")
                nc.gpsimd.collective_compute(
                    "AllGather", mybir.AluOpType.bypass,
                    ins=[gin.opt()], outs=[gout.opt()],
                    replica_groups=[list(range(NCORES))])
                chs = wk.tile([P, KC, NMAXC], BF16, tag="chs")
                for g in range(NCORES):
                    eng = nc.sync if g % 2 == 0 else nc.scalar
                    eng.dma_start(chs[:, :, g * nmax:(g + 1) * nmax], gout[g])

                # fh matmuls overlap the AG flight (independent of ch_sum)
                psf = pg.tile([P, NCHMX], F32, tag="psf")
                for k in range(KC):
                    nc.tensor.matmul(
                        psf[:, :nch], fhs[:, k, :],
                        hch[:, k, hch_off:hch_off + nch],
                        start=(k == 0), stop=(k == KC - 1))

                psi = pg.tile([P, 3, P], F32, tag="psi")
                for g in range(3):
                    for k in range(KC):
                        nc.tensor.matmul(
                            psi[:, g, :ncols], iouhs[:, k * 3 + g, :],
                            chs[:, k, :ncols],
                            start=(k == 0), stop=(k == KC - 1))

                n = ncols
                tmp = wk.tile([P, 3, NMAXC], F32, tag="gtmp")
                nc.vector.tensor_add(tmp[:, :, :n], psi[:, :, :n],
                                     xi_f[:, :, base:base + n])
                ig = wk.tile([P, NMAXC], F32, tag="ig")
                og = wk.tile([P, NMAXC], F32, tag="og")
                ug = wk.tile([P, NMAXC], F32, tag="ug")
                nc.scalar.activation(ig[:, :n], tmp[:, 0, :n], ACT.Sigmoid,
                                     bias=biou[:, 0:1])
                nc.scalar.activation(og[:, :n], tmp[:, 1, :n], ACT.Sigmoid,
                                     bias=biou[:, 1:2])
                nc.scalar.activation(ug[:, :n], tmp[:, 2, :n], ACT.Tanh,
                                     bias=biou[:, 2:3])
                cn = wk.tile([P, NMAXC], F32, tag="cn")
                nc.vector.tensor_mul(cn[:, :n], ig[:, :n], ug[:, :n])
                fsb = wk.tile([P, NCHMX], F32, tag="fsb")
                xfb = wk.tile([P, NCHMX], F32, tag="xfb")
                xfb_v = xfb[:, :nch].rearrange("p (n k) -> p n k", k=K)
                for kk in range(K):
                    nc.vector.tensor_copy(
                        xfb_v[:, :, kk:kk + 1],
                        xf_f[:, base:base + n].rearrange(
                            "p (n one) -> p n one", one=1))
                nc.vector.tensor_add(fsb[:, :nch], psf[:, :nch], xfb[:, :nch])
                nc.scalar.activation(fsb[:, :nch], fsb[:, :nch],
                                     ACT.Sigmoid, bias=bfh[:, 0:1])
                nc.vector.tensor_mul(fsb[:, :nch], fsb[:, :nch], ccg_ap)
                fc = wk.tile([P, NMAXC], F32, tag="fc")
                nc.vector.tensor_reduce(
                    fc[:, :n],
                    fsb[:, :nch].rearrange("p (n k) -> p n k", k=K),
                    axis=mybir.AxisListType.X, op=mybir.AluOpType.add)
                nc.vector.tensor_add(cn[:, :n], cn[:, :n], fc[:, :n])
                nc.vector.tensor_copy(c_sl[:, base:base + n], cn[:, :n])
                tct = wk.tile([P, NMAXC], F32, tag="tct")
                nc.scalar.activation(tct[:, :n], cn[:, :n], ACT.Tanh)
                nc.vector.tensor_mul(h_sl[:, base:base + n], og[:, :n],
                                     tct[:, :n])
                publish(base, ncols, str(wi + 1),
                        c_extra=info.get("extras"))

            # ---- replicated-full top waves ---------------------------------
            for info in wave_info:
                if info["kind"] != "replf":
                    continue
                base, ncols = info["base"], info["ncols"]
                nch = K * ncols
                foff, cpos0 = info["foff"], info["cpos0"]

                hchF = wk.tile([P, KC, K * REPLF_MAX], BF16, tag="hchF")
                ccgF = wk.tile([P, KC, K * REPLF_MAX], BF16, tag="ccgF")
                if info["has_missing"]:
                    nc.vector.memset(hchF[:, :, :nch], 0.0)
                    nc.vector.memset(ccgF[:, :, :nch], 0.0)
                for (dst, src, ln) in info["runs"]:
                    nc.vector.tensor_copy(hchF[:, :, dst:dst + ln],
                                          h_bf[:, :, src:src + ln])
                for (dst, src, ln) in info["cruns"]:
                    nc.vector.tensor_copy(ccgF[:, :, dst:dst + ln],
                                          cfull[:, :, src:src + ln])
                hsF = wk.tile([P, KC, REPLF_MAX], F32, tag="hsF")
                nc.vector.tensor_reduce(
                    hsF[:, :, :ncols],
                    hchF[:, :, :nch].rearrange("p k (n c) -> p k n c", c=K),
                    axis=mybir.AxisListType.X, op=mybir.AluOpType.add)
                hsFb = wk.tile([P, KC, REPLF_MAX], BF16, tag="hsFb")
                nc.vector.tensor_copy(hsFb[:, :, :ncols], hsF[:, :, :ncols])

                # replicated relation matmuls: per-rel contiguous col ranges
                psR = pp.tile([P, KC, P], F32, tag="psr")
                for m in range(KC):
                    for si, (r, lo, hi) in enumerate(info["ranges"]):
                        wst = wtile[info["soff"] + si]
                        for k in range(KC):
                            nc.tensor.matmul(
                                psR[:, m, lo:hi], wst[:, m * KC + k, :],
                                hsFb[:, k, lo:hi],
                                start=(k == 0), stop=(k == KC - 1))
                chsF = wk.tile([P, KC, REPLF_MAX], BF16, tag="chsF")
                nc.scalar.activation(chsF[:, :, :ncols], psR[:, :, :ncols],
                                     ACT.Identity,
                                     scale=1.0 / (WSCALE * WSCALE))

                # full fh (x WSCALE)
                psfF = pg.tile([P, KC, K * REPLF_MAX], F32, tag="psfF")
                for m in range(KC):
                    for k in range(KC):
                        nc.tensor.matmul(
                            psfF[:, m, :nch], fhFt[:, m * KC + k, :],
                            hchF[:, k, :nch],
                            start=(k == 0), stop=(k == KC - 1))

                # full iou
                psiF = pg.tile([P, 3 * KC, REPLF_MAX], F32, tag="psiF")
                for mg in range(3 * KC):
                    for k in range(KC):
                        nc.tensor.matmul(
                            psiF[:, mg, :ncols], iouhFt[:, mg * KC + k, :],
                            chsF[:, k, :ncols],
                            start=(k == 0), stop=(k == KC - 1))

                tmpF = wk.tile([P, 3 * KC, REPLF_MAX], F32, tag="tmpF")
                nc.vector.tensor_add(tmpF[:, :, :ncols], psiF[:, :, :ncols],
                                     xiFt[:, :, foff:foff + ncols])
                igF = wk.tile([P, KC, REPLF_MAX], F32, tag="igF")
                ogF = wk.tile([P, KC, REPLF_MAX], F32, tag="ogF")
                ugF = wk.tile([P, KC, REPLF_MAX], F32, tag="ugF")
                nc.scalar.activation(igF[:, :, :ncols],
                                     tmpF[:, 0:KC, :ncols], ACT.Sigmoid)
                nc.scalar.activation(ogF[:, :, :ncols],
                                     tmpF[:, KC:2 * KC, :ncols], ACT.Sigmoid)
                nc.scalar.activation(ugF[:, :, :ncols],
                                     tmpF[:, 2 * KC:3 * KC, :ncols], ACT.Tanh)
                cnF = wk.tile([P, KC, REPLF_MAX], F32, tag="cnF")
                nc.vector.tensor_mul(cnF[:, :, :ncols], igF[:, :, :ncols],
                                     ugF[:, :, :ncols])
                # f = sigmoid((psfF + WSCALE*(xf+b)) / WSCALE)
                fsbF = wk.tile([P, KC, K * REPLF_MAX], F32, tag="fsbF")
                xfbF = wk.tile([P, KC, K * REPLF_MAX], F32, tag="xfbF")
                for m in range(KC):
                    xfb_v = xfbF[:, m, :nch].rearrange("p (n k) -> p n k", k=K)
                    for kk in range(K):
                        nc.vector.tensor_copy(
                            xfb_v[:, :, kk:kk + 1],
                            xffFt[:, m, foff:foff + ncols].rearrange(
                                "p (n one) -> p n one", one=1))
                nc.vector.tensor_add(fsbF[:, :, :nch], psfF[:, :, :nch],
                                     xfbF[:, :, :nch])
                nc.scalar.activation(fsbF[:, :, :nch], fsbF[:, :, :nch],
                                     ACT.Sigmoid, scale=1.0 / WSCALE)
                nc.vector.tensor_mul(fsbF[:, :, :nch], fsbF[:, :, :nch],
                                     ccgF[:, :, :nch])
                fcF = wk.tile([P, KC, REPLF_MAX], F32, tag="fcF")
                nc.vector.tensor_reduce(
                    fcF[:, :, :ncols],
                    fsbF[:, :, :nch].rearrange("p m (n k) -> p m n k", k=K),
                    axis=mybir.AxisListType.X, op=mybir.AluOpType.add)
                nc.vector.tensor_add(cnF[:, :, :ncols], cnF[:, :, :ncols],
                                     fcF[:, :, :ncols])
                thF = wk.tile([P, KC, REPLF_MAX], F32, tag="thF")
                nc.scalar.activation(thF[:, :, :ncols], cnF[:, :, :ncols],
                                     ACT.Tanh)
                hF = wk.tile([P, KC, REPLF_MAX], F32, tag="hF")
                nc.vector.tensor_mul(hF[:, :, :ncols], ogF[:, :, :ncols],
                                     thF[:, :, :ncols])
                # update replicated state
                nc.vector.tensor_copy(h_bf[:, :, base:base + ncols],
                                      hF[:, :, :ncols])
                nc.vector.tensor_copy(cfull[:, :, cpos0:cpos0 + ncols],
                                      cnF[:, :, :ncols])
                # extract own feature slice via select mask
                sh = wk.tile([P, KC, REPLF_MAX], F32, tag="selh")
                nc.vector.tensor_mul(sh[:, :, :ncols], hF[:, :, :ncols],
                                     selmt[:, :, foff:foff + ncols])
                nc.vector.tensor_reduce(
                    h_sl[:, base:base + ncols],
                    sh[:, :, :ncols].rearrange("p k n -> p n k"),
                    axis=mybir.AxisListType.X, op=mybir.AluOpType.add)

            nc.sync.dma_start(d_hout.ap(), h_sl[:])

    def _pmajor(a):
        return np.ascontiguousarray(a.transpose(1, 0, 2))

    iouhF_t = _pmajor(iouhF)
    fhF_t = _pmajor(fhF)
    xiF_t = _pmajor(xiF)
    xffF_t = _pmajor(xffF)
    in_maps = []
    for c in range(NCORES):
        in_maps.append({
            "wstream": wstream[c], "masks": maskbuf[c],
            "xt": xT_b, "iouxstat": _pmajor(iouxstat[c]),
            "iouhstat": _pmajor(iouhstat[c]),
            "fxstat": _pmajor(fxstat[c]), "fhstat": _pmajor(fhstat[c]),
            "iouhF": iouhF_t, "fhF": fhF_t, "xiF": xiF_t, "xffF": xffF_t,
            "selm": selm[c],
            "b_xi": b_xi[c], "b_iou": b_iou[c], "b_xf": b_xf[c],
            "b_fh": b_fh[c],
        })
    _split_multi_waits(nc)
    kernel._nc = nc
    kernel._in_maps = in_maps
    res = run_bass_kernel_spmd(nc, in_maps, list(range(NCORES)))
    hT = np.concatenate([res.results[c]["hout"] for c in range(NCORES)], 0)
    out = np.empty((N, MEM), np.float32)
    for node in range(N):
        out[node] = hT[:, col_of[node]]
    return out
